# revision 1
# baseline (speedup 1.0000x reference)
"""Trainium2 Bass kernel for a dense transformer encoder layer.

Problem: B=4, S=2048, D=768, H=12 heads (DH=64), FFN 3072, fp32 I/O.

Sharding (no collectives): 8 cores = (batch b, sequence half) pairs.
Each core computes the full layer for its 1024 query rows; K/V projections
for the full 2048-row sequence of its batch are duplicated across the two
cores sharing a batch (cheaper than collectives here).

Layout strategy: all activations are kept feature-major ("xT" = [D, S]) so
every matmul uses native weight slices as the stationary operand and
feature-major activations as the moving operand; the attention core runs
with scoresT = [keys, q] so no on-chip transposes are ever needed. Inputs
are transposed/staged host-side (layout prep is part of sharding).

Numerics: attention + projections fully bf16, FFN bf16 weights/activations
with f32 PSUM accumulation, residual/LN fp32. Softmax needs no
max-subtraction (logits ~ +-3). Softmax denominators are free: each head's
V stationary carries a ones column ([vA(64),1 | vB(64),1] per head-pair,
130 cols), so the denominator accumulates in PSUM row 64 of the ctx
matmul; normalization uses DMA partition-broadcast (no PE/ACT involved).

Schedule: phases are software-pipelined so PE-heavy work hides under the
ACT-bound softmax stretches: V projection overlaps the first head-pair of
qc0 attention; out-proj + LayerNorm of qc0 overlap qc1 attention.
"""
from contextlib import ExitStack

import numpy as np
import ml_dtypes

import concourse.bass as bass
import concourse.tile as tile
from concourse import bacc, mybir
from concourse.bass_utils import run_bass_kernel_spmd

FR = mybir.dt.float32r
F32 = mybir.dt.float32
BF = mybir.dt.bfloat16
AF = mybir.ActivationFunctionType
OP = mybir.AluOpType

B, S, D, H = 4, 2048, 768, 12
DH, DF = 64, 3072
SQ = 1024            # query rows per core
NK = D // 128        # 6 feature chunks
NF = DF // 128       # 24 ffn chunks
KC = S // 128        # 16 key chunks
NQ = SQ // 512       # 2 query column chunks
HP = H // 2          # 6 head pairs
PW = 130             # cols per head pair in vp: [vA(64), 1, vB(64), 1]
NT = 8               # FFN weight slices
MF = 3               # dF 128-chunks per slice
DT = DF // NT        # 384 cols per W1 slice
N_CORES = 8
SCALE = 1.0 / 8.0    # 1/sqrt(DH)
EPS = 1e-5

GELU_FUNC = AF.Gelu

# bias pack layout (columns in "sp" [128, 828])
_BQ, _BK, _BO, _B2, _LNG, _LNB, _B1, _BV = 0, 6, 12, 18, 24, 30, 36, 60


def _body(nc, tc, io):
    xqb_d, xqf_d, xk_d, xv_d = io["xqb"], io["xqf"], io["xk"], io["xv"]
    wq_d, wk_d, wv_d, wo_d = io["wq"], io["wk"], io["wv"], io["wo"]
    w1_d, w2_d, sp_d = io["w1"], io["w2"], io["sp"]
    ones_fr_d, out_d = io["ones_fr"], io["out"]

    r6 = lambda ap: ap.rearrange("(c p) s -> p c s", p=128)

    with ExitStack() as ctx:
        Po = lambda **kw: ctx.enter_context(tc.tile_pool(**kw))
        const = Po(name="const", bufs=1)
        sb = Po(name="sb", bufs=1)

        sp = const.tile([128, 828], F32)
        nc.sync.dma_start(out=sp[:], in_=sp_d)
        ones_fr = const.tile([128, 128], FR)
        nc.sync.dma_start(out=ones_fr[:], in_=ones_fr_d)
        sel = const.tile([128, 128], FR)
        nc.sync.dma_start(out=sel[:], in_=io["sel"])
        bias = lambda idx, j: sp[:, idx + j : idx + j + 1]

        # shared weight slots: 4 x 9KB
        def wtile(name, ncols, dt):
            return sb.tile([128, ncols], dt, tag="w", bufs=4, name=name)

        wk = wtile("wk", NK * D, BF)
        nc.sync.dma_start(out=wk[:].rearrange("p (c m) -> p c m", m=D), in_=r6(wk_d))
        wq = wtile("wq", NK * D, BF)
        nc.sync.dma_start(out=wq[:].rearrange("p (c m) -> p c m", m=D), in_=r6(wq_d))
        wv = wtile("wv", NK * D, BF)
        nc.sync.dma_start(out=wv[:].rearrange("p (c m) -> p c m", m=D), in_=r6(wv_d))

        # persistent activations
        kpT = sb.tile([128, NK * S], BF, tag="kpx", name="kpT")
        qpT = sb.tile([128, NK * SQ], BF, tag="qpT", name="qpT")
        vp = sb.tile([128, KC * HP * PW], BF, tag="vph", name="vp")
        ctxS = sb.tile([128, NK * SQ], BF, tag="ctxS", name="ctxS")
        x_sb = sb.tile([128, NK * SQ], FR, tag="xsb", name="x_sb")
        hT = sb.tile([128, NK * SQ], BF, tag="vph", name="hT")

        # vp viewed [p, key-chunk, head-pair, head(2), 65]; ones at col 64
        vp5 = vp[:].rearrange("p (s j g w) -> p s j g w", j=HP, g=2, w=65)
        nc.gpsimd.memset(vp5[:, :, :, :, 64:65], 1.0)
        bv4 = sp[:, _BV : _BV + D].rearrange("p (j g c) -> p j g c", g=2, c=64)

        # ---------------- emission helpers ----------------
        def proj_T(pool, dst, w, xd, sc, bidx, scols):
            """dst[mc-chunk, sc-block] = W.T @ x for one 512-col block."""
            xt = sb.tile([128, NK * 512], BF, tag="xu", bufs=3, name="xt")
            nc.sync.dma_start(
                out=xt[:].rearrange("p (c s) -> p c s", s=512),
                in_=r6(xd)[:, :, sc * 512 : (sc + 1) * 512])
            for mc in range(NK):
                ps = pool.tile([128, 512], F32, tag="pa", name="psp")
                for kc in range(NK):
                    nc.tensor.matmul(
                        ps[:],
                        w[:, kc * D + mc * 128 : kc * D + (mc + 1) * 128],
                        xt[:, kc * 512 : (kc + 1) * 512],
                        start=(kc == 0), stop=(kc == NK - 1))
                nc.scalar.activation(
                    dst[:, mc * scols + sc * 512 : mc * scols + (sc + 1) * 512],
                    ps[:], AF.Identity, bias=bias(bidx, mc), scale=1.0)

        xv_tiles = {}

        def vp_block(pool, srow):
            """vp[srow] = xv[:, srow-chunk].T @ Wv (+bv), pair layout."""
            sc, m = divmod(srow, 4)
            if sc not in xv_tiles:
                xv_t = sb.tile([128, NK * 512], BF, tag="xu", bufs=3,
                               name="xv_t")
                nc.sync.dma_start(
                    out=xv_t[:].rearrange("p (c s) -> p c s", s=512),
                    in_=r6(xv_d)[:, :, sc * 512 : (sc + 1) * 512])
                xv_tiles.clear()
                xv_tiles[sc] = xv_t
            xv_t = xv_tiles[sc]
            for n0, nsz in ((0, 512), (512, 256)):
                j0, nj = n0 // 128, nsz // 128
                ps = pool.tile([128, 512], F32, tag="pa", name="psv")
                for kc in range(NK):
                    nc.tensor.matmul(
                        ps[:, :nsz],
                        xv_t[:, kc * 512 + m * 128 : kc * 512 + (m + 1) * 128],
                        wv[:, kc * D + n0 : kc * D + n0 + nsz],
                        start=(kc == 0), stop=(kc == NK - 1),
                        skip_group_check=True)
                psv = ps[:, :nsz].rearrange("p (j g c) -> p j g c", g=2, c=64)
                nc.vector.tensor_add(
                    vp5[:, srow, j0 : j0 + nj, 0, 0:64],
                    psv[:, :, 0, :], bv4[:, j0 : j0 + nj, 0, :])
                nc.vector.tensor_add(
                    vp5[:, srow, j0 : j0 + nj, 1, 0:64],
                    psv[:, :, 1, :], bv4[:, j0 : j0 + nj, 1, :])

        def scores_exp(sc_ps, qc, j, kc):
            psAB = sc_ps.tile([128, 1024], F32, tag="sc", name="psAB")
            nc.tensor.matmul(
                psAB[:, 0:512],
                kpT[0:64, j * S + kc * 128 : j * S + (kc + 1) * 128],
                qpT[0:64, j * SQ + qc * 512 : j * SQ + (qc + 1) * 512],
                start=True, stop=True, skip_group_check=True)
            nc.tensor.matmul(
                psAB[:, 512:1024],
                kpT[64:128, j * S + kc * 128 : j * S + (kc + 1) * 128],
                qpT[64:128, j * SQ + qc * 512 : j * SQ + (qc + 1) * 512],
                start=True, stop=True, skip_group_check=True)
            eAB = sb.tile([128, 1024], BF, tag="ex", bufs=3, name="eAB")
            nc.scalar.activation(eAB[:], psAB[:], AF.Exp, scale=SCALE)
            return eAB

        def ctx_pair(ctxA, ctxB, eAB, j, kc):
            pb = kc * HP * PW + j * PW
            nc.tensor.matmul(
                ctxA[0:65, :], vp[:, pb : pb + 65], eAB[:, 0:512],
                start=(kc == 0), stop=(kc == KC - 1), skip_group_check=True)
            nc.tensor.matmul(
                ctxB[0:65, :], vp[:, pb + 65 : pb + PW], eAB[:, 512:1024],
                start=(kc == 0), stop=(kc == KC - 1), skip_group_check=True)

        def epilogue(sc_ps, ctxA, ctxB, qc, j):
            # den recips on rows 64 (head A) / 32 (head B), then broadcast
            # across partitions via a sel-matmul into a scores-pool slot.
            r_rec = sb.tile([128, 512], FR, tag="dn", bufs=1, name="r_rec")
            with nc.allow_low_precision(reason="softmax recip"):
                nc.vector.reciprocal(r_rec[64:65, :], ctxA[64:65, :])
                nc.vector.reciprocal(r_rec[32:33, :], ctxB[64:65, :])
            rb64 = sc_ps.tile([64, 1024], F32, tag="sc", name="rb64")
            nc.tensor.matmul(rb64[:, 0:512], sel[64:65, 0:64],
                             r_rec[64:65, :], start=True, stop=True,
                             skip_group_check=True)
            nc.tensor.matmul(rb64[:, 512:1024], sel[32:33, 64:128],
                             r_rec[32:33, :], start=True, stop=True,
                             skip_group_check=True)
            rb_s = sb.tile([128, 512], F32, tag="rbs", bufs=1, name="rb_s")
            nc.vector.tensor_copy(rb_s[0:64, :], rb64[:, 0:512])
            nc.vector.tensor_copy(rb_s[64:128, :], rb64[:, 512:1024])
            ccols = slice(j * SQ + qc * 512, j * SQ + (qc + 1) * 512)
            with nc.allow_low_precision(reason="bf16 ctx"):
                nc.vector.tensor_mul(
                    ctxS[0:64, ccols], ctxA[0:64, :], rb_s[0:64, :])
                nc.vector.tensor_mul(
                    ctxS[64:128, ccols], ctxB[0:64, :], rb_s[64:128, :])

        def b_iter(sc_ps, cx_ps, qc, j):
            ctxA = cx_ps.tile([128, 512], F32, tag="cx", name="ctxA")
            ctxB = cx_ps.tile([128, 512], F32, tag="cx", name="ctxB")
            for kc in range(KC):
                eAB = scores_exp(sc_ps, qc, j, kc)
                ctx_pair(ctxA, ctxB, eAB, j, kc)
            epilogue(sc_ps, ctxA, ctxB, qc, j)

        def c1_chunk(pc, qc, mc):
            """out-proj + residual for one feature chunk."""
            ps = pc.tile([128, 512], F32, tag="pc", name="pso")
            for kc in range(NK):
                nc.tensor.matmul(
                    ps[:],
                    wo[:, kc * D + mc * 128 : kc * D + (mc + 1) * 128],
                    ctxS[:, kc * SQ + qc * 512 : kc * SQ + (qc + 1) * 512],
                    start=(kc == 0), stop=(kc == NK - 1),
                    skip_group_check=True)
            xqc = sb.tile([128, 512], F32, tag="sm", bufs=2, name="xqc")
            nc.sync.dma_start(
                out=xqc[:], in_=r6(xqf_d)[:, mc, qc * 512 : (qc + 1) * 512])
            with nc.allow_low_precision(reason="f32r residual"):
                nc.vector.scalar_tensor_tensor(
                    x_sb[:, mc * SQ + qc * 512 : mc * SQ + (qc + 1) * 512],
                    ps[:], bias(_BO, mc), xqc[:], OP.add, OP.add)

        def c2_stats(stp, qc):
            """LayerNorm stats for one 512-query block (PE/DVE only)."""
            mean_t = stp.tile([1, 512], F32, tag="st", name="mean")
            var_t = stp.tile([1, 512], F32, tag="st", name="var")
            for kc in range(NK):
                xcols = slice(kc * SQ + qc * 512, kc * SQ + (qc + 1) * 512)
                xsq = sb.tile([128, 512], FR, tag="sq", bufs=1, name="xsq")
                with nc.allow_low_precision(reason="f32r x^2 for LN var"):
                    nc.vector.tensor_mul(xsq[:], x_sb[:, xcols], x_sb[:, xcols])
                nc.tensor.matmul(
                    mean_t[:], ones_fr[:, 0:1], x_sb[:, xcols],
                    start=(kc == 0), stop=(kc == NK - 1),
                    skip_group_check=True)
                nc.tensor.matmul(
                    var_t[:], ones_fr[:, 0:1], xsq[:],
                    start=(kc == 0), stop=(kc == NK - 1),
                    skip_group_check=True)
            return mean_t, var_t

        def c2_apply(stp, qc, mean_t, var_t):
            """LayerNorm normalize+affine into hT (uses ACT Sqrt)."""
            mu = sb.tile([1, 512], F32, tag="r1", bufs=2, name="mu")
            e2 = sb.tile([1, 512], F32, tag="r2", bufs=2, name="e2")
            sd = sb.tile([1, 512], F32, tag="r3", bufs=2, name="sd")
            rs = sb.tile([1, 512], FR, tag="r4", bufs=2, name="rs")
            mrs = sb.tile([1, 512], FR, tag="r5", bufs=2, name="mrs")
            nc.vector.tensor_scalar_mul(mu[:], mean_t[:], 1.0 / D)
            nc.vector.tensor_scalar_mul(e2[:], var_t[:], 1.0 / D)
            nc.vector.tensor_mul(sd[:], mu[:], mu[:])
            nc.vector.tensor_sub(e2[:], e2[:], sd[:])
            nc.vector.tensor_scalar_add(e2[:], e2[:], EPS)
            nc.scalar.activation(sd[:], e2[:], AF.Sqrt)
            with nc.allow_low_precision(reason="f32r LN rows"):
                nc.vector.reciprocal(rs[:], sd[:])
                nc.vector.tensor_mul(mrs[:], mu[:].bitcast(FR), rs[:])
            A_p = stp.tile([128, 512], F32, tag="st", name="A_p")
            B_p = stp.tile([128, 512], F32, tag="st", name="B_p")
            nc.tensor.matmul(A_p[:], ones_fr[0:1, :], rs[:],
                             start=True, stop=True, skip_group_check=True)
            nc.tensor.matmul(B_p[:], ones_fr[0:1, :], mrs[:],
                             start=True, stop=True, skip_group_check=True)
            for kc in range(NK):
                xcols = slice(kc * SQ + qc * 512, kc * SQ + (qc + 1) * 512)
                t1 = sb.tile([128, 512], F32, tag="sm", bufs=2, name="t1")
                with nc.allow_low_precision(reason="f32r LN mul"):
                    nc.vector.tensor_mul(t1[:], x_sb[:, xcols], A_p[:])
                t2 = sb.tile([128, 512], F32, tag="sm", bufs=2, name="t2")
                with nc.allow_low_precision(reason="f32r LN sub"):
                    nc.vector.tensor_sub(t2[:], t1[:], B_p[:])
                with nc.allow_low_precision(reason="bf16 hT"):
                    nc.scalar.activation(
                        hT[:, xcols], t2[:], AF.Identity,
                        bias=bias(_LNB, kc), scale=bias(_LNG, kc))

        # ---------------- schedule ----------------
        with (tc.tile_pool(name="sc_ps", bufs=2, space="PSUM") as sc_ps,
              tc.tile_pool(name="cx_ps", bufs=2, space="PSUM") as cx_ps):
            with tc.tile_pool(name="pa", bufs=2, space="PSUM") as pa:
                # A1: K/Q projections
                for sc in range(S // 512):
                    proj_T(pa, kpT, wk, xk_d, sc, _BK, S)
                for sc in range(NQ):
                    proj_T(pa, qpT, wq, xqb_d, sc, _BQ, SQ)
                wo = wtile("wo", NK * D, BF)
                nc.sync.dma_start(
                    out=wo[:].rearrange("p (c m) -> p c m", m=D), in_=r6(wo_d))
                # A2 (V projection) interleaved with B(qc0, j0)
                ctxA0 = cx_ps.tile([128, 512], F32, tag="cx", name="ctxA")
                ctxB0 = cx_ps.tile([128, 512], F32, tag="cx", name="ctxB")
                eprev = None
                for kc in range(KC):
                    vp_block(pa, kc)
                    e = scores_exp(sc_ps, 0, 0, kc)
                    if kc >= 1:
                        ctx_pair(ctxA0, ctxB0, eprev, 0, kc - 1)
                    eprev = e
                ctx_pair(ctxA0, ctxB0, eprev, 0, KC - 1)
                epilogue(sc_ps, ctxA0, ctxB0, 0, 0)
            for j in range(1, HP):
                b_iter(sc_ps, cx_ps, 0, j)
            # B(qc1) with C1(qc0)/C2(qc0)-stats interleaved
            with tc.tile_pool(name="pc", bufs=2, space="PSUM") as pc:
                for j in range(0, 3):
                    b_iter(sc_ps, cx_ps, 1, j)
                    c1_chunk(pc, 0, 2 * j)
                    c1_chunk(pc, 0, 2 * j + 1)
            with tc.tile_pool(name="st", bufs=2, space="PSUM") as stp:
                b_iter(sc_ps, cx_ps, 1, 3)
                mv0 = c2_stats(stp, 0)
                b_iter(sc_ps, cx_ps, 1, 4)
                b_iter(sc_ps, cx_ps, 1, 5)
                c2_apply(stp, 0, *mv0)

        # C1/C2 for qc1
        with tc.tile_pool(name="pc2", bufs=2, space="PSUM") as pc:
            for mc in range(NK):
                c1_chunk(pc, 1, mc)
        with tc.tile_pool(name="st2", bufs=2, space="PSUM") as stp:
            mv1 = c2_stats(stp, 1)
            c2_apply(stp, 1, *mv1)

        # ---------------- phase D: FFN ----------------
        with (tc.tile_pool(name="ff_ps", bufs=6, space="PSUM") as ff_ps,
              tc.tile_pool(name="u_ps", bufs=2, space="PSUM") as u_ps):
            for sc in range(NQ):
                ffp = [ff_ps.tile([128, 512], F32, tag="ff", name=f"ffp{i}")
                       for i in range(NK)]
                for t in range(NT):
                    w1t = wtile(f"w1_{sc}_{t}", NK * DT, BF)
                    nc.sync.dma_start(
                        out=w1t[:].rearrange("p (c m) -> p c m", m=DT),
                        in_=w1_d.rearrange("(c p) (t m) -> p c t m",
                                           p=128, t=NT)[:, :, t, :])
                    w2t = wtile(f"w2_{sc}_{t}", MF * D, BF)
                    nc.sync.dma_start(
                        out=w2t[:].rearrange("p (c m) -> p c m", m=D),
                        in_=w2_d.rearrange("(t c p) m -> p t c m",
                                           p=128, c=MF)[:, t])
                    ut = sb.tile([128, MF * 512], BF, tag="xu", bufs=3, name="ut")
                    for mf in range(MF):
                        up = u_ps.tile([128, 512], F32, tag="up", name="up")
                        for kc in range(NK):
                            nc.tensor.matmul(
                                up[:],
                                w1t[:, kc * DT + mf * 128 : kc * DT + (mf + 1) * 128],
                                hT[:, kc * SQ + sc * 512 : kc * SQ + (sc + 1) * 512],
                                start=(kc == 0), stop=(kc == NK - 1))
                        with nc.allow_low_precision(reason="bf16 u"):
                            nc.scalar.activation(
                                ut[:, mf * 512 : (mf + 1) * 512], up[:],
                                GELU_FUNC, bias=bias(_B1, t * MF + mf),
                                scale=1.0)
                    for md in range(NK):
                        for c in range(MF):
                            nc.tensor.matmul(
                                ffp[md][:],
                                w2t[:, c * D + md * 128 : c * D + (md + 1) * 128],
                                ut[:, c * 512 : (c + 1) * 512],
                                start=(t == 0 and c == 0),
                                stop=(t == NT - 1 and c == MF - 1),
                                skip_group_check=True)
                for md in range(NK):
                    ot = sb.tile([128, 512], F32, tag="sm", bufs=2, name="ot")
                    nc.scalar.activation(ot[:], ffp[md][:], AF.Identity,
                                         bias=bias(_B2, md), scale=1.0)
                    nc.sync.dma_start(
                        out=r6(out_d)[:, md, sc * 512 : (sc + 1) * 512],
                        in_=ot[:])


def _build(reps=1):
    nc = bacc.Bacc("TRN2", target_bir_lowering=False, debug=False,
                   num_devices=N_CORES)
    io = {
        "xqb": nc.dram_tensor("xqb", [D, SQ], BF, kind="ExternalInput").ap(),
        "xqf": nc.dram_tensor("xqf", [D, SQ], F32, kind="ExternalInput").ap(),
        "xk": nc.dram_tensor("xk", [D, S], BF, kind="ExternalInput").ap(),
        "xv": nc.dram_tensor("xv", [D, S], BF, kind="ExternalInput").ap(),
        "wq": nc.dram_tensor("wq", [D, D], BF, kind="ExternalInput").ap(),
        "wk": nc.dram_tensor("wk", [D, D], BF, kind="ExternalInput").ap(),
        "wv": nc.dram_tensor("wv", [D, D], BF, kind="ExternalInput").ap(),
        "wo": nc.dram_tensor("wo", [D, D], BF, kind="ExternalInput").ap(),
        "w1": nc.dram_tensor("w1", [D, DF], BF, kind="ExternalInput").ap(),
        "w2": nc.dram_tensor("w2", [DF, D], BF, kind="ExternalInput").ap(),
        "sp": nc.dram_tensor("sp", [128, 828], F32, kind="ExternalInput").ap(),
        "ones_fr": nc.dram_tensor("ones_fr", [128, 128], FR,
                                  kind="ExternalInput").ap(),
        "sel": nc.dram_tensor("sel", [128, 128], FR, kind="ExternalInput").ap(),
        "out": nc.dram_tensor("out", [D, SQ], F32, kind="ExternalOutput").ap(),
    }
    with tile.TileContext(nc) as tc:
        if reps == 1:
            _body(nc, tc, io)
        else:
            with tc.For_i(0, reps, 1):
                _body(nc, tc, io)
    nc.compile()
    return nc


_NC = None


def _get_nc():
    global _NC
    if _NC is None:
        _NC = _build()
    return _NC


def make_in_maps(inputs):
    """Shard + lay out the full inputs for the 8 cores (numpy only)."""
    f = lambda k: np.asarray(inputs[k], np.float32)
    bf = lambda a: np.ascontiguousarray(a).astype(ml_dtypes.bfloat16)
    Q, K, V = f("Q"), f("K"), f("V")
    sp = np.zeros((128, 828), np.float32)
    for idx, key in ((_BQ, "bq"), (_BK, "bk"), (_BO, "bo"), (_B2, "b2"),
                     (_LNG, "ln_g"), (_LNB, "ln_b")):
        sp[:, idx : idx + NK] = f(key).reshape(NK, 128).T
    sp[:, _B1 : _B1 + NF] = f("b1").reshape(NF, 128).T
    sp[:, _BV : _BV + D] = np.broadcast_to(f("bv"), (128, D))
    shared = {
        "wq": bf(f("Wq")), "wk": bf(f("Wk")), "wv": bf(f("Wv")),
        "wo": bf(f("Wo")), "w1": bf(f("W1")), "w2": bf(f("W2")), "sp": sp,
        "ones_fr": np.ones((128, 128), np.float32),
        "sel": _sel_matrix(),
    }
    in_maps = []
    for c in range(N_CORES):
        b, half = divmod(c, 2)
        r0 = half * SQ
        xqf = np.ascontiguousarray(Q[b, r0 : r0 + SQ, :].T)
        in_maps.append(dict(
            shared,
            xqb=xqf.astype(ml_dtypes.bfloat16),
            xqf=xqf,
            xk=bf(K[b].T),
            xv=bf(V[b].T),
        ))
    return in_maps


def _sel_matrix():
    # rb broadcast rows: row 64 -> rb partitions 0:64 (head A),
    # row 32 -> partitions 64:128 (head B)
    sel = np.zeros((128, 128), np.float32)
    sel[64, 0:64] = 1.0
    sel[32, 64:128] = 1.0
    return sel


def assemble(results):
    out = np.empty((B, S, D), np.float32)
    for c in range(N_CORES):
        b, half = divmod(c, 2)
        r0 = half * SQ
        out[b, r0 : r0 + SQ, :] = results[c]["out"].T
    return out


def kernel(**inputs):
    nc = _get_nc()
    res = run_bass_kernel_spmd(nc, make_in_maps(inputs), list(range(N_CORES)))
    return assemble(res.results)



# revision 9
# speedup vs baseline: 1.1034x; 1.1034x over previous
"""Trainium2 Bass kernel for a dense transformer encoder layer.

Problem: B=4, S=2048, D=768, H=12 heads (DH=64), FFN 3072, fp32 I/O.

Sharding (no collectives): 8 cores = (batch b, sequence half) pairs.
Each core computes the full layer for its 1024 query rows; K/V projections
for the full 2048-row sequence of its batch are duplicated across the two
cores sharing a batch (cheaper than collectives here).

Layout strategy: all activations are kept feature-major ("xT" = [D, S]) so
every matmul uses native weight slices as the stationary operand and
feature-major activations as the moving operand; the attention core runs
with scoresT = [keys, q] so no on-chip transposes are ever needed. Inputs
are transposed/staged host-side (layout prep is part of sharding).

Numerics: attention + projections fully bf16, FFN bf16 weights/activations
with f32 PSUM accumulation, residual/LN fp32. Softmax needs no
max-subtraction (logits ~ +-3). Softmax denominators are free: each head's
V stationary carries a ones column ([vA(64),1 | vB(64),1] per head-pair,
130 cols), so the denominator accumulates in PSUM row 64 of the ctx
matmul; normalization uses DMA partition-broadcast (no PE/ACT involved).

Schedule: phases are software-pipelined so PE-heavy work hides under the
ACT-bound softmax stretches: V projection overlaps the first head-pair of
qc0 attention; out-proj + LayerNorm of qc0 overlap qc1 attention.
"""
from contextlib import ExitStack

import numpy as np
import ml_dtypes

import concourse.bass as bass
import concourse.tile as tile
from concourse import bacc, mybir
from concourse.bass_utils import run_bass_kernel_spmd

FR = mybir.dt.float32r
F32 = mybir.dt.float32
BF = mybir.dt.bfloat16
FP8 = mybir.dt.float8e4
PM = mybir.MatmulPerfMode.DoubleRow
AF = mybir.ActivationFunctionType
OP = mybir.AluOpType
WSC = 16.0           # host-side prescale on fp8 Q/K/V weights

B, S, D, H = 4, 2048, 768, 12
DH, DF = 64, 3072
SQ = 1024            # query rows per core
NK = D // 128        # 6 feature chunks
NF = DF // 128       # 24 ffn chunks
KC = S // 128        # 16 key chunks
NQ = SQ // 512       # 2 query column chunks
HP = H // 2          # 6 head pairs
PW = 130             # cols per head pair in vp: [vA(64), 1, vB(64), 1]
NT = 8               # FFN weight slices
MF = 3               # dF 128-chunks per slice
DT = DF // NT        # 384 cols per W1 slice
N_CORES = 8
SCALE = 1.0 / 8.0    # 1/sqrt(DH)
EPS = 1e-5

GELU_FUNC = AF.Gelu

# bias pack layout (columns in "sp" [128, 828])
_BQ, _BK, _BO, _B2, _LNG, _LNB, _B1, _BV = 0, 6, 12, 18, 24, 30, 36, 60


def _body(nc, tc, io):
    xqb_d, xqf_d, xk_d, xv_d = io["xqb"], io["xqf"], io["xk"], io["xv"]
    wq_d, wk_d, wv_d, wo_d = io["wq"], io["wk"], io["wv"], io["wo"]
    w1_d, w2_d, sp_d = io["w1"], io["w2"], io["sp"]
    ones_fr_d, out_d = io["ones_fr"], io["out"]

    r6 = lambda ap: ap.rearrange("(c p) s -> p c s", p=128)

    with ExitStack() as ctx:
        Po = lambda **kw: ctx.enter_context(tc.tile_pool(**kw))
        const = Po(name="const", bufs=1)
        sb = Po(name="sb", bufs=1)

        sp = const.tile([128, 828], F32)
        nc.sync.dma_start(out=sp[:], in_=sp_d)
        ones_fr = const.tile([128, 128], FR)
        nc.sync.dma_start(out=ones_fr[:], in_=ones_fr_d)
        sel = const.tile([128, 128], FR)
        nc.sync.dma_start(out=sel[:], in_=io["sel"])
        bias = lambda idx, j: sp[:, idx + j : idx + j + 1]

        # shared weight slots: 4 x 9KB
        def wtile(name, ncols, dt):
            return sb.tile([128, ncols], dt, tag="w", bufs=4, name=name)

        wk = wtile("wk", NK * D, FP8)
        nc.sync.dma_start(out=wk[:].rearrange("p (c m) -> p c m", m=D), in_=r6(wk_d))
        wq = wtile("wq", NK * D, FP8)
        nc.sync.dma_start(out=wq[:].rearrange("p (c m) -> p c m", m=D), in_=r6(wq_d))
        wv = wtile("wv", NK * D, FP8)
        nc.sync.dma_start(out=wv[:].rearrange("p (c m) -> p c m", m=D), in_=r6(wv_d))

        # persistent activations
        kpT = sb.tile([128, NK * S], BF, tag="kpx", name="kpT")
        qpT = sb.tile([128, NK * SQ], BF, tag="qpT", name="qpT")
        vp = sb.tile([128, KC * HP * PW], BF, tag="vph", name="vp")
        ctxS = sb.tile([128, NK * SQ], BF, tag="ctxS", name="ctxS")
        x_sb = sb.tile([128, NK * SQ], FR, tag="xsb", name="x_sb")
        hT = sb.tile([128, NK * SQ], BF, tag="vph", name="hT")

        # vp viewed [p, key-chunk, head-pair, head(2), 65]; ones at col 64
        vp5 = vp[:].rearrange("p (s j g w) -> p s j g w", j=HP, g=2, w=65)
        nc.gpsimd.memset(vp5[:, :, :, :, 64:65], 1.0)
        bv4 = sp[:, _BV : _BV + D].rearrange("p (j g c) -> p j g c", g=2, c=64)

        # ---------------- emission helpers ----------------
        def proj_T(pool, dst, w, xd, sc, bidx, scols):
            """dst[mc-chunk, sc-block] = W.T @ x for one 512-col block.

            fp8e4 DoubleRow: contract 768 as 3 pairs of 128-row k-tiles at
            0.5 cycles/row; weights carry a x16 host prescale, undone by the
            activation's scale."""
            xt = sb.tile([128, NK * 512], FP8, tag="xu", bufs=3, name="xt")
            nc.sync.dma_start(
                out=xt[:].rearrange("p (c s) -> p c s", s=512),
                in_=r6(xd)[:, :, sc * 512 : (sc + 1) * 512])
            w3 = w[:].rearrange("p (c m) -> p c m", m=D)
            x3 = xt[:].rearrange("p (c s) -> p c s", s=512)
            for mc in range(NK):
                ps = pool.tile([128, 512], F32, tag="pa", name="psp")
                for kp in range(NK // 2):
                    nc.tensor.matmul(
                        ps[:],
                        w3[:, 2 * kp : 2 * kp + 2, mc * 128 : (mc + 1) * 128],
                        x3[:, 2 * kp : 2 * kp + 2, :],
                        start=(kp == 0), stop=(kp == NK // 2 - 1),
                        perf_mode=PM)
                nc.scalar.activation(
                    dst[:, mc * scols + sc * 512 : mc * scols + (sc + 1) * 512],
                    ps[:], AF.Identity, bias=bias(bidx, mc), scale=1.0 / WSC)

        xv_tiles = {}

        def vp_block(pool, srow):
            """vp[srow] = xv[:, srow-chunk].T @ Wv (+bv), pair layout."""
            sc, m = divmod(srow, 4)
            if sc not in xv_tiles:
                xv_t = sb.tile([128, NK * 512], FP8, tag="xu", bufs=3,
                               name="xv_t")
                nc.sync.dma_start(
                    out=xv_t[:].rearrange("p (c s) -> p c s", s=512),
                    in_=r6(xv_d)[:, :, sc * 512 : (sc + 1) * 512])
                xv_tiles.clear()
                xv_tiles[sc] = xv_t
            xv_t = xv_tiles[sc]
            xv3 = xv_t[:].rearrange("p (c s) -> p c s", s=512)
            wv3 = wv[:].rearrange("p (c m) -> p c m", m=D)
            for n0, nsz in ((0, 512), (512, 256)):
                j0, nj = n0 // 128, nsz // 128
                ps = pool.tile([128, 512], F32, tag="pa", name="psv")
                for kp in range(NK // 2):
                    nc.tensor.matmul(
                        ps[:, :nsz],
                        xv3[:, 2 * kp : 2 * kp + 2,
                            m * 128 : (m + 1) * 128],
                        wv3[:, 2 * kp : 2 * kp + 2, n0 : n0 + nsz],
                        start=(kp == 0), stop=(kp == NK // 2 - 1),
                        perf_mode=PM, skip_group_check=True)
                psv = ps[:, :nsz].rearrange("p (j g c) -> p j g c", g=2, c=64)
                with nc.allow_low_precision(reason="fp8 wv descale"):
                    nc.vector.scalar_tensor_tensor(
                        vp5[:, srow, j0 : j0 + nj, 0, 0:64],
                        psv[:, :, 0, :], 1.0 / WSC,
                        bv4[:, j0 : j0 + nj, 0, :], OP.mult, OP.add)
                    nc.vector.scalar_tensor_tensor(
                        vp5[:, srow, j0 : j0 + nj, 1, 0:64],
                        psv[:, :, 1, :], 1.0 / WSC,
                        bv4[:, j0 : j0 + nj, 1, :], OP.mult, OP.add)

        def scores_exp(sc_ps, qc, j, kc):
            psAB = sc_ps.tile([128, 1024], F32, tag="sc", name="psAB")
            nc.tensor.matmul(
                psAB[:, 0:512],
                kpT[0:64, j * S + kc * 128 : j * S + (kc + 1) * 128],
                qpT[0:64, j * SQ + qc * 512 : j * SQ + (qc + 1) * 512],
                start=True, stop=True, skip_group_check=True)
            nc.tensor.matmul(
                psAB[:, 512:1024],
                kpT[64:128, j * S + kc * 128 : j * S + (kc + 1) * 128],
                qpT[64:128, j * SQ + qc * 512 : j * SQ + (qc + 1) * 512],
                start=True, stop=True, skip_group_check=True)
            eAB = sb.tile([128, 1024], BF, tag="ex", bufs=3, name="eAB")
            nc.scalar.activation(eAB[:], psAB[:], AF.Exp, scale=SCALE)
            return eAB

        def ctx_pair(ctxA, ctxB, eAB, j, kc):
            pb = kc * HP * PW + j * PW
            nc.tensor.matmul(
                ctxA[0:65, :], vp[:, pb : pb + 65], eAB[:, 0:512],
                start=(kc == 0), stop=(kc == KC - 1), skip_group_check=True)
            nc.tensor.matmul(
                ctxB[0:65, :], vp[:, pb + 65 : pb + PW], eAB[:, 512:1024],
                start=(kc == 0), stop=(kc == KC - 1), skip_group_check=True)

        def epilogue(sc_ps, ctxA, ctxB, qc, j):
            # den recips on rows 64 (head A) / 32 (head B), then broadcast
            # across partitions via a sel-matmul into a scores-pool slot.
            r_rec = sb.tile([128, 512], FR, tag="dn", bufs=1, name="r_rec")
            with nc.allow_low_precision(reason="softmax recip"):
                nc.vector.reciprocal(r_rec[64:65, :], ctxA[64:65, :])
                nc.vector.reciprocal(r_rec[32:33, :], ctxB[64:65, :])
            rb64 = sc_ps.tile([64, 1024], F32, tag="sc", name="rb64")
            nc.tensor.matmul(rb64[:, 0:512], sel[64:65, 0:64],
                             r_rec[64:65, :], start=True, stop=True,
                             skip_group_check=True)
            nc.tensor.matmul(rb64[:, 512:1024], sel[32:33, 64:128],
                             r_rec[32:33, :], start=True, stop=True,
                             skip_group_check=True)
            rb_s = sb.tile([128, 512], F32, tag="rbs", bufs=1, name="rb_s")
            nc.vector.tensor_copy(rb_s[0:64, :], rb64[:, 0:512])
            nc.vector.tensor_copy(rb_s[64:128, :], rb64[:, 512:1024])
            ccols = slice(j * SQ + qc * 512, j * SQ + (qc + 1) * 512)
            with nc.allow_low_precision(reason="bf16 ctx"):
                nc.vector.tensor_mul(
                    ctxS[0:64, ccols], ctxA[0:64, :], rb_s[0:64, :])
                nc.vector.tensor_mul(
                    ctxS[64:128, ccols], ctxB[0:64, :], rb_s[64:128, :])

        def b_iter(sc_ps, cx_ps, qc, j):
            ctxA = cx_ps.tile([128, 512], F32, tag="cx", name="ctxA")
            ctxB = cx_ps.tile([128, 512], F32, tag="cx", name="ctxB")
            for kc in range(KC):
                eAB = scores_exp(sc_ps, qc, j, kc)
                ctx_pair(ctxA, ctxB, eAB, j, kc)
            epilogue(sc_ps, ctxA, ctxB, qc, j)

        def c1_chunk(pc, qc, mc):
            """out-proj + residual for one feature chunk."""
            ps = pc.tile([128, 512], F32, tag="pc", name="pso")
            for kc in range(NK):
                nc.tensor.matmul(
                    ps[:],
                    wo[:, kc * D + mc * 128 : kc * D + (mc + 1) * 128],
                    ctxS[:, kc * SQ + qc * 512 : kc * SQ + (qc + 1) * 512],
                    start=(kc == 0), stop=(kc == NK - 1),
                    skip_group_check=True)
            xqc = sb.tile([128, 512], F32, tag="sm", bufs=2, name="xqc")
            nc.sync.dma_start(
                out=xqc[:], in_=r6(xqf_d)[:, mc, qc * 512 : (qc + 1) * 512])
            with nc.allow_low_precision(reason="f32r residual"):
                nc.vector.scalar_tensor_tensor(
                    x_sb[:, mc * SQ + qc * 512 : mc * SQ + (qc + 1) * 512],
                    ps[:], bias(_BO, mc), xqc[:], OP.add, OP.add)

        def c2_stats(stp, qc):
            """LayerNorm stats for one 512-query block (PE/DVE only)."""
            mean_t = stp.tile([1, 512], F32, tag="st", name="mean")
            var_t = stp.tile([1, 512], F32, tag="st", name="var")
            for kc in range(NK):
                xcols = slice(kc * SQ + qc * 512, kc * SQ + (qc + 1) * 512)
                xsq = sb.tile([128, 512], FR, tag="sq", bufs=1, name="xsq")
                with nc.allow_low_precision(reason="f32r x^2 for LN var"):
                    nc.vector.tensor_mul(xsq[:], x_sb[:, xcols], x_sb[:, xcols])
                nc.tensor.matmul(
                    mean_t[:], ones_fr[:, 0:1], x_sb[:, xcols],
                    start=(kc == 0), stop=(kc == NK - 1),
                    skip_group_check=True)
                nc.tensor.matmul(
                    var_t[:], ones_fr[:, 0:1], xsq[:],
                    start=(kc == 0), stop=(kc == NK - 1),
                    skip_group_check=True)
            return mean_t, var_t

        def c2_apply(stp, qc, mean_t, var_t):
            """LayerNorm normalize+affine into hT (uses ACT Sqrt)."""
            mu = sb.tile([1, 512], F32, tag="r1", bufs=2, name="mu")
            e2 = sb.tile([1, 512], F32, tag="r2", bufs=2, name="e2")
            sd = sb.tile([1, 512], F32, tag="r3", bufs=2, name="sd")
            rs = sb.tile([1, 512], FR, tag="r4", bufs=2, name="rs")
            mrs = sb.tile([1, 512], FR, tag="r5", bufs=2, name="mrs")
            nc.vector.tensor_scalar_mul(mu[:], mean_t[:], 1.0 / D)
            nc.vector.tensor_scalar_mul(e2[:], var_t[:], 1.0 / D)
            nc.vector.tensor_mul(sd[:], mu[:], mu[:])
            nc.vector.tensor_sub(e2[:], e2[:], sd[:])
            nc.vector.tensor_scalar_add(e2[:], e2[:], EPS)
            nc.scalar.activation(sd[:], e2[:], AF.Sqrt)
            with nc.allow_low_precision(reason="f32r LN rows"):
                nc.vector.reciprocal(rs[:], sd[:])
                nc.vector.tensor_mul(mrs[:], mu[:].bitcast(FR), rs[:])
            A_p = stp.tile([128, 512], F32, tag="st", name="A_p")
            B_p = stp.tile([128, 512], F32, tag="st", name="B_p")
            nc.tensor.matmul(A_p[:], ones_fr[0:1, :], rs[:],
                             start=True, stop=True, skip_group_check=True)
            nc.tensor.matmul(B_p[:], ones_fr[0:1, :], mrs[:],
                             start=True, stop=True, skip_group_check=True)
            for kc in range(NK):
                xcols = slice(kc * SQ + qc * 512, kc * SQ + (qc + 1) * 512)
                t1 = sb.tile([128, 512], F32, tag="sm", bufs=2, name="t1")
                with nc.allow_low_precision(reason="f32r LN mul"):
                    nc.vector.tensor_mul(t1[:], x_sb[:, xcols], A_p[:])
                t2 = sb.tile([128, 512], F32, tag="sm", bufs=2, name="t2")
                with nc.allow_low_precision(reason="f32r LN sub"):
                    nc.vector.tensor_sub(t2[:], t1[:], B_p[:])
                with nc.allow_low_precision(reason="bf16 hT"):
                    nc.scalar.activation(
                        hT[:, xcols], t2[:], AF.Identity,
                        bias=bias(_LNB, kc), scale=bias(_LNG, kc))

        # ---------------- schedule ----------------
        with (tc.tile_pool(name="sc_ps", bufs=2, space="PSUM") as sc_ps,
              tc.tile_pool(name="cx_ps", bufs=2, space="PSUM") as cx_ps):
            with tc.tile_pool(name="pa", bufs=2, space="PSUM") as pa:
                # A1: K/Q projections
                for sc in range(S // 512):
                    proj_T(pa, kpT, wk, xk_d, sc, _BK, S)
                for sc in range(NQ):
                    proj_T(pa, qpT, wq, xqb_d, sc, _BQ, SQ)
                wo = wtile("wo", NK * D, BF)
                nc.sync.dma_start(
                    out=wo[:].rearrange("p (c m) -> p c m", m=D), in_=r6(wo_d))
                # A2 (V projection) interleaved with B(qc0, j0)
                ctxA0 = cx_ps.tile([128, 512], F32, tag="cx", name="ctxA")
                ctxB0 = cx_ps.tile([128, 512], F32, tag="cx", name="ctxB")
                eprev = None
                for kc in range(KC):
                    vp_block(pa, kc)
                    e = scores_exp(sc_ps, 0, 0, kc)
                    if kc >= 1:
                        ctx_pair(ctxA0, ctxB0, eprev, 0, kc - 1)
                    eprev = e
                ctx_pair(ctxA0, ctxB0, eprev, 0, KC - 1)
                epilogue(sc_ps, ctxA0, ctxB0, 0, 0)
            for j in range(1, HP):
                b_iter(sc_ps, cx_ps, 0, j)
            # B(qc1) with C1(qc0)/C2(qc0)-stats interleaved
            with tc.tile_pool(name="pc", bufs=2, space="PSUM") as pc:
                for j in range(0, 3):
                    b_iter(sc_ps, cx_ps, 1, j)
                    c1_chunk(pc, 0, 2 * j)
                    c1_chunk(pc, 0, 2 * j + 1)
            with tc.tile_pool(name="st", bufs=2, space="PSUM") as stp:
                b_iter(sc_ps, cx_ps, 1, 3)
                mv0 = c2_stats(stp, 0)
                b_iter(sc_ps, cx_ps, 1, 4)
                b_iter(sc_ps, cx_ps, 1, 5)
                c2_apply(stp, 0, *mv0)

        # C1/C2 for qc1
        with tc.tile_pool(name="pc2", bufs=2, space="PSUM") as pc:
            for mc in range(NK):
                c1_chunk(pc, 1, mc)
        with tc.tile_pool(name="st2", bufs=2, space="PSUM") as stp:
            mv1 = c2_stats(stp, 1)
            c2_apply(stp, 1, *mv1)

        # ---------------- phase D: FFN ----------------
        with (tc.tile_pool(name="ff_ps", bufs=6, space="PSUM") as ff_ps,
              tc.tile_pool(name="u_ps", bufs=2, space="PSUM") as u_ps):
            for sc in range(NQ):
                ffp = [ff_ps.tile([128, 512], F32, tag="ff", name=f"ffp{i}")
                       for i in range(NK)]
                for t in range(NT):
                    w1t = wtile(f"w1_{sc}_{t}", NK * DT, BF)
                    nc.sync.dma_start(
                        out=w1t[:].rearrange("p (c m) -> p c m", m=DT),
                        in_=w1_d.rearrange("(c p) (t m) -> p c t m",
                                           p=128, t=NT)[:, :, t, :])
                    w2t = wtile(f"w2_{sc}_{t}", MF * D, BF)
                    nc.sync.dma_start(
                        out=w2t[:].rearrange("p (c m) -> p c m", m=D),
                        in_=w2_d.rearrange("(t c p) m -> p t c m",
                                           p=128, c=MF)[:, t])
                    ut = sb.tile([128, MF * 512], BF, tag="xu", bufs=3, name="ut")
                    for mf in range(MF):
                        up = u_ps.tile([128, 512], F32, tag="up", name="up")
                        for kc in range(NK):
                            nc.tensor.matmul(
                                up[:],
                                w1t[:, kc * DT + mf * 128 : kc * DT + (mf + 1) * 128],
                                hT[:, kc * SQ + sc * 512 : kc * SQ + (sc + 1) * 512],
                                start=(kc == 0), stop=(kc == NK - 1))
                        with nc.allow_low_precision(reason="bf16 u"):
                            nc.scalar.activation(
                                ut[:, mf * 512 : (mf + 1) * 512], up[:],
                                GELU_FUNC, bias=bias(_B1, t * MF + mf),
                                scale=1.0)
                    for md in range(NK):
                        for c in range(MF):
                            nc.tensor.matmul(
                                ffp[md][:],
                                w2t[:, c * D + md * 128 : c * D + (md + 1) * 128],
                                ut[:, c * 512 : (c + 1) * 512],
                                start=(t == 0 and c == 0),
                                stop=(t == NT - 1 and c == MF - 1),
                                skip_group_check=True)
                for md in range(NK):
                    ot = sb.tile([128, 512], F32, tag="sm", bufs=2, name="ot")
                    nc.scalar.activation(ot[:], ffp[md][:], AF.Identity,
                                         bias=bias(_B2, md), scale=1.0)
                    nc.sync.dma_start(
                        out=r6(out_d)[:, md, sc * 512 : (sc + 1) * 512],
                        in_=ot[:])


def _build(reps=1):
    nc = bacc.Bacc("TRN2", target_bir_lowering=False, debug=False,
                   num_devices=N_CORES)
    io = {
        "xqb": nc.dram_tensor("xqb", [D, SQ], FP8, kind="ExternalInput").ap(),
        "xqf": nc.dram_tensor("xqf", [D, SQ], F32, kind="ExternalInput").ap(),
        "xk": nc.dram_tensor("xk", [D, S], FP8, kind="ExternalInput").ap(),
        "xv": nc.dram_tensor("xv", [D, S], FP8, kind="ExternalInput").ap(),
        "wq": nc.dram_tensor("wq", [D, D], FP8, kind="ExternalInput").ap(),
        "wk": nc.dram_tensor("wk", [D, D], FP8, kind="ExternalInput").ap(),
        "wv": nc.dram_tensor("wv", [D, D], FP8, kind="ExternalInput").ap(),
        "wo": nc.dram_tensor("wo", [D, D], BF, kind="ExternalInput").ap(),
        "w1": nc.dram_tensor("w1", [D, DF], BF, kind="ExternalInput").ap(),
        "w2": nc.dram_tensor("w2", [DF, D], BF, kind="ExternalInput").ap(),
        "sp": nc.dram_tensor("sp", [128, 828], F32, kind="ExternalInput").ap(),
        "ones_fr": nc.dram_tensor("ones_fr", [128, 128], FR,
                                  kind="ExternalInput").ap(),
        "sel": nc.dram_tensor("sel", [128, 128], FR, kind="ExternalInput").ap(),
        "out": nc.dram_tensor("out", [D, SQ], F32, kind="ExternalOutput").ap(),
    }
    with tile.TileContext(nc) as tc:
        if reps == 1:
            _body(nc, tc, io)
        else:
            with tc.For_i(0, reps, 1):
                _body(nc, tc, io)
    nc.compile()
    return nc


_NC = None


def _get_nc():
    global _NC
    if _NC is None:
        _NC = _build()
    return _NC


def make_in_maps(inputs):
    """Shard + lay out the full inputs for the 8 cores (numpy only)."""
    f = lambda k: np.asarray(inputs[k], np.float32)
    bf = lambda a: np.ascontiguousarray(a).astype(ml_dtypes.bfloat16)
    f8 = lambda a: np.ascontiguousarray(a).astype(ml_dtypes.float8_e4m3)
    Q, K, V = f("Q"), f("K"), f("V")
    sp = np.zeros((128, 828), np.float32)
    for idx, key in ((_BQ, "bq"), (_BK, "bk"), (_BO, "bo"), (_B2, "b2"),
                     (_LNG, "ln_g"), (_LNB, "ln_b")):
        sp[:, idx : idx + NK] = f(key).reshape(NK, 128).T
    sp[:, _B1 : _B1 + NF] = f("b1").reshape(NF, 128).T
    sp[:, _BV : _BV + D] = np.broadcast_to(f("bv"), (128, D))
    shared = {
        "wq": f8(WSC * f("Wq")), "wk": f8(WSC * f("Wk")),
        "wv": f8(WSC * f("Wv")),
        "wo": bf(f("Wo")), "w1": bf(f("W1")), "w2": bf(f("W2")), "sp": sp,
        "ones_fr": np.ones((128, 128), np.float32),
        "sel": _sel_matrix(),
    }
    in_maps = []
    for c in range(N_CORES):
        b, half = divmod(c, 2)
        r0 = half * SQ
        xqf = np.ascontiguousarray(Q[b, r0 : r0 + SQ, :].T)
        in_maps.append(dict(
            shared,
            xqb=xqf.astype(ml_dtypes.float8_e4m3),
            xqf=xqf,
            xk=f8(K[b].T),
            xv=f8(V[b].T),
        ))
    return in_maps


def _sel_matrix():
    # rb broadcast rows: row 64 -> rb partitions 0:64 (head A),
    # row 32 -> partitions 64:128 (head B)
    sel = np.zeros((128, 128), np.float32)
    sel[64, 0:64] = 1.0
    sel[32, 64:128] = 1.0
    return sel


def assemble(results):
    out = np.empty((B, S, D), np.float32)
    for c in range(N_CORES):
        b, half = divmod(c, 2)
        r0 = half * SQ
        out[b, r0 : r0 + SQ, :] = results[c]["out"].T
    return out


def kernel(**inputs):
    nc = _get_nc()
    res = run_bass_kernel_spmd(nc, make_in_maps(inputs), list(range(N_CORES)))
    return assemble(res.results)

